# revision 1
# baseline (speedup 1.0000x reference)
"""Causal self-attention Trainium2 kernel (8 NeuronCores).

Sharding (Megatron-style, per sharding_hint):
  core c -> batch b = c//2, head-group g = c%2 (8 of 16 heads).
  W_q/W_k/W_v column-sliced per head group; W_o row-sliced; host sums the
  two partial outputs per batch (tensor-parallel reduce) and adds b_o.

Per-core kernel (all matmuls bf16 with fp32 PSUM accumulation):
  xT    [1024, 2048]  x[b] transposed (d_emb on partitions)
  wqkv  [1024, 1536]  [Wq_g | Wk_g | Wv_g]
  wo    [512, 1024]   W_o rows for this head group
  out   [2048, 1024]  fp32 partial (no bias)

Layouts: qT/kT stored [head_dim, n] so score matmuls contract over the
64-dim head axis; the two heads of a partition-tile occupy partitions
0:64 / 64:128, and their score matmuls are emitted interleaved so the PE
runs them concurrently in different row groups. Scores are computed
TRANSPOSED ([k, q]) so the exp'd weights feed the ctx matmul directly as
the moving operand; v is kept [n, head_dim] with a ones-block per head so
a single matmul yields both ctx^T and the softmax denominators broadcast
across 64 partitions.

Causal handling: k-tiles strictly above the diagonal are skipped; on
diagonal sub-tiles the fully-masked query prefix is never computed
(scores and ctx matmuls trim their moving operand to q >= k-block start),
and only the 128x128 diagonal block gets a triangular bf16 multiply.

`reps` repeats the whole body inside one NEFF — used only for timing
((T(n)-T(1))/(n-1) cancels dispatch overhead); the graded path is reps=1.
"""

import sys

import numpy as np

sys.path.insert(0, "/opt/trn_rl_repo")

import ml_dtypes

BF16 = ml_dtypes.bfloat16

D_EMB = 1024
N_SEQ = 2048
N_HEADS_CORE = 8  # heads per core
HD = 64  # head dim
KT = D_EMB // 128  # 8 k-tiles over d_emb
PT = 4  # partition tiles over the 512 per-core head dims
NT = N_SEQ // 128  # 16 n-tiles
QC = N_SEQ // 512  # 4 query chunks of 512
SCALE = 1.0 / np.sqrt(HD)

_CACHE = {}


def _emit_body(nc, tc, mybir, sfx, xT_d, wqkv_d, wo_d, out_d):
    f32 = mybir.dt.float32
    bf16 = mybir.dt.bfloat16

    with tc.tile_pool(name=f"persist{sfx}", bufs=1) as persist:
        wo_sb = [
            persist.tile([128, D_EMB], bf16, name=f"wo{p}{sfx}", tag=f"wo{p}")
            for p in range(PT)
        ]
        qt_sb = [
            persist.tile([128, N_SEQ], bf16, name=f"qt{p}{sfx}", tag=f"qt{p}")
            for p in range(PT)
        ]
        kt_sb = [
            persist.tile([128, N_SEQ], bf16, name=f"kt{p}{sfx}", tag=f"kt{p}")
            for p in range(PT)
        ]
        ctxt_sb = [
            persist.tile([128, N_SEQ], bf16, name=f"ctxt{p}{sfx}", tag=f"ctxt{p}")
            for p in range(PT)
        ]
        # v per n-tile [128, 1024]: head h -> cols h*128:h*128+64 = v_h,
        # cols h*128+64:h*128+128 = 1.0 (softmax denominator ones-trick)
        v_sb = [
            persist.tile([128, 1024], bf16, name=f"v{nt}{sfx}", tag=f"v{nt}")
            for nt in range(NT)
        ]
        tri_sb = persist.tile([128, 128], bf16, name=f"tri{sfx}", tag="tri")

        def vaug_ap(nt, h):
            return v_sb[nt][:, h * 128 : (h + 1) * 128]

        # ---- constants (gpsimd, no deps) ----
        for nt in range(NT):
            ones_view = v_sb[nt].rearrange("p (h c) -> p h c", h=N_HEADS_CORE)
            nc.gpsimd.memset(ones_view[:, :, 64:128], 1.0)
        # tri[k_local, q_local] = 1.0 if q_local >= k_local else 0
        nc.gpsimd.memset(tri_sb[:], 1.0)
        nc.gpsimd.affine_select(
            out=tri_sb[:],
            in_=tri_sb[:],
            compare_op=mybir.AluOpType.is_ge,
            fill=0.0,
            base=0,
            pattern=[[1, 128]],
            channel_multiplier=-1,
        )

        with tc.tile_pool(name=f"xw{sfx}", bufs=1) as xw_pool:
            xt_sb = [
                xw_pool.tile([128, N_SEQ], bf16, name=f"xt{k}{sfx}", tag=f"xt{k}")
                for k in range(KT)
            ]
            wqkv_sb = [
                xw_pool.tile([128, 1536], bf16, name=f"wqkv{k}{sfx}", tag=f"wqkv{k}")
                for k in range(KT)
            ]

            # ---- input DMA, split across queues for parallel load ----
            for k in range(KT):
                nc.sync.dma_start(
                    out=xt_sb[k][:], in_=xT_d[k * 128 : (k + 1) * 128, :]
                )
                nc.gpsimd.dma_start(
                    out=wqkv_sb[k][:], in_=wqkv_d[k * 128 : (k + 1) * 128, :]
                )
            for p in range(PT):
                nc.sync.dma_start(
                    out=wo_sb[p][:], in_=wo_d[p * 128 : (p + 1) * 128, :]
                )

            with tc.tile_pool(name=f"psq{sfx}", bufs=3, space="PSUM") as psq_pool:
                # ---- v = x @ Wv  ([n, 512] per n-tile) ----
                for nt in range(NT):
                    psv = psq_pool.tile(
                        [128, 512], f32, name=f"psv{nt}{sfx}", tag="psv"
                    )
                    for k in range(KT):
                        nc.tensor.matmul(
                            psv[:],
                            lhsT=xt_sb[k][:, nt * 128 : (nt + 1) * 128],
                            rhs=wqkv_sb[k][:, 1024:1536],
                            start=(k == 0),
                            stop=(k == KT - 1),
                        )
                    v_view = v_sb[nt].rearrange("p (h c) -> p h c", h=N_HEADS_CORE)
                    nc.vector.tensor_copy(
                        v_view[:, :, 0:64],
                        psv.rearrange("p (h c) -> p h c", h=N_HEADS_CORE),
                    )

                # ---- qT, kT = (x @ Wq)^T, (x @ Wk)^T  [hd, n] layout ----
                for qn in range(QC):
                    nsl = slice(qn * 512, (qn + 1) * 512)
                    for p in range(PT):
                        for which, dst in ((0, qt_sb), (1, kt_sb)):
                            ps = psq_pool.tile(
                                [128, 512],
                                f32,
                                name=f"psqk{p}_{qn}_{which}{sfx}",
                                tag="psqk",
                            )
                            col0 = which * 512 + p * 128
                            for k in range(KT):
                                nc.tensor.matmul(
                                    ps[:],
                                    lhsT=wqkv_sb[k][:, col0 : col0 + 128],
                                    rhs=xt_sb[k][:, nsl],
                                    start=(k == 0),
                                    stop=(k == KT - 1),
                                )
                            nc.vector.tensor_copy(dst[p][:, nsl], ps[:])

        # ---- attention (qc outer so out-proj can follow each chunk) ----
        with (
            tc.tile_pool(name=f"expp{sfx}", bufs=4) as expp,
            tc.tile_pool(name=f"rpool{sfx}", bufs=4) as rpool,
            tc.tile_pool(name=f"outp{sfx}", bufs=3) as outp,
            tc.tile_pool(name=f"pssc{sfx}", bufs=2, space="PSUM") as pssc_pool,
            tc.tile_pool(name=f"psctx{sfx}", bufs=2, space="PSUM") as psctx_pool,
        ):
            for qc in range(QC):
                q0 = qc * 512
                nk = 4 * qc + 4  # causal: k-tiles 0..nk-1
                ngroups = nk // 2
                for p in range(PT):
                    # both heads' ctx in one 2-bank tile: h2 -> cols h2*512
                    ctx_ps = psctx_pool.tile(
                        [128, 1024], f32, name=f"ctx{p}_{qc}{sfx}", tag="ctx"
                    )
                    for gi in range(ngroups):
                        ps = [
                            pssc_pool.tile(
                                [128, 1024],
                                f32,
                                name=f"sc{p}_{qc}_{gi}_{h2}{sfx}",
                                tag="sc",
                            )
                            for h2 in range(2)
                        ]
                        ex = [
                            expp.tile(
                                [128, 1024],
                                bf16,
                                name=f"ex{p}_{qc}_{gi}_{h2}{sfx}",
                                tag="ex",
                            )
                            for h2 in range(2)
                        ]
                        # interleave heads so PE overlaps the row-group pairs
                        for j in range(2):
                            ki = 2 * gi + j
                            jj = ki - 4 * qc  # >=0 on diagonal sub-tiles
                            t0 = max(0, 128 * jj)  # masked-prefix trim
                            for h2 in range(2):
                                hb = h2 * 64
                                nc.tensor.matmul(
                                    ps[h2][:, j * 512 + t0 : (j + 1) * 512],
                                    lhsT=kt_sb[p][
                                        hb : hb + 64, ki * 128 : (ki + 1) * 128
                                    ],
                                    rhs=qt_sb[p][hb : hb + 64, q0 + t0 : q0 + 512],
                                    start=True,
                                    stop=True,
                                )
                        for h2 in range(2):
                            nc.scalar.activation(
                                ex[h2][:],
                                ps[h2][:],
                                mybir.ActivationFunctionType.Exp,
                                scale=float(SCALE),
                            )
                        for j in range(2):
                            ki = 2 * gi + j
                            jj = ki - 4 * qc
                            if jj >= 0:  # triangular block on the diagonal
                                blk = slice(
                                    j * 512 + 128 * jj, j * 512 + 128 * jj + 128
                                )
                                for h2 in range(2):
                                    nc.vector.tensor_mul(
                                        ex[h2][:, blk], ex[h2][:, blk], tri_sb[:]
                                    )
                        for j in range(2):
                            ki = 2 * gi + j
                            jj = ki - 4 * qc
                            t0 = max(0, 128 * jj)
                            for h2 in range(2):
                                h = 2 * p + h2
                                nc.tensor.matmul(
                                    ctx_ps[:, h2 * 512 + t0 : (h2 + 1) * 512],
                                    lhsT=vaug_ap(ki, h),
                                    rhs=ex[h2][:, j * 512 + t0 : (j + 1) * 512],
                                    start=(ki == 0),
                                    stop=(ki == nk - 1),
                                )
                    rec = rpool.tile(
                        [64, 1024], f32, name=f"rec{p}_{qc}{sfx}", tag="rec"
                    )
                    nc.vector.reciprocal(rec[:], ctx_ps[64:128, :])
                    for h2 in range(2):
                        nc.vector.tensor_mul(
                            ctxt_sb[p][h2 * 64 : h2 * 64 + 64, q0 : q0 + 512],
                            ctx_ps[0:64, h2 * 512 : (h2 + 1) * 512],
                            rec[:, h2 * 512 : (h2 + 1) * 512],
                        )

                # ---- out = ctx @ Wo for this chunk's n-tiles ----
                for nt in range(4 * qc, 4 * qc + 4):
                    # reuse the score pool's slots (free between chunks)
                    pso = pssc_pool.tile(
                        [128, 1024], f32, name=f"pso{nt}{sfx}", tag="sc"
                    )
                    for dh in range(2):
                        for p in range(PT):
                            nc.tensor.matmul(
                                pso[:, dh * 512 : (dh + 1) * 512],
                                lhsT=ctxt_sb[p][:, nt * 128 : (nt + 1) * 128],
                                rhs=wo_sb[p][:, dh * 512 : (dh + 1) * 512],
                                start=(p == 0),
                                stop=(p == PT - 1),
                            )
                    osb = outp.tile(
                        [128, 1024], f32, name=f"osb{nt}{sfx}", tag="osb"
                    )
                    nc.vector.tensor_copy(osb[:], pso[:])
                    nc.sync.dma_start(
                        out=out_d[nt * 128 : (nt + 1) * 128, :], in_=osb[:]
                    )


def _build_module(reps=1):
    import concourse.bacc as bacc
    import concourse.mybir as mybir
    import concourse.tile as tile

    f32 = mybir.dt.float32
    bf16 = mybir.dt.bfloat16

    nc = bacc.Bacc()
    xT_d = nc.dram_tensor("xT", [D_EMB, N_SEQ], bf16, kind="ExternalInput")
    wqkv_d = nc.dram_tensor("wqkv", [D_EMB, 1536], bf16, kind="ExternalInput")
    wo_d = nc.dram_tensor("wo", [512, D_EMB], bf16, kind="ExternalInput")
    out_d = nc.dram_tensor("out", [N_SEQ, D_EMB], f32, kind="ExternalOutput")

    with tile.TileContext(nc) as tc:
        for rep in range(reps):
            _emit_body(
                nc, tc, mybir, f"_r{rep}" if reps > 1 else "",
                xT_d, wqkv_d, wo_d, out_d,
            )

    if not nc.is_finalized():
        nc.finalize()
    return nc


def _get_module(reps=1):
    key = f"nc{reps}"
    if key not in _CACHE:
        _CACHE[key] = _build_module(reps)
    return _CACHE[key]


def make_in_maps(x, W_q, W_k, W_v, W_o):
    in_maps = []
    for c in range(8):
        b, g = c // 2, c % 2
        gs = slice(g * 512, (g + 1) * 512)
        xT = np.ascontiguousarray(x[b].T).astype(BF16)
        wqkv = np.concatenate(
            [W_q[:, gs], W_k[:, gs], W_v[:, gs]], axis=1
        ).astype(BF16)
        wo = np.ascontiguousarray(W_o[gs, :]).astype(BF16)
        in_maps.append({"xT": xT, "wqkv": wqkv, "wo": wo})
    return in_maps


def kernel(x, W_q, W_k, W_v, W_o, b_o):
    from concourse.bass_utils import run_bass_kernel_spmd

    nc = _get_module()
    in_maps = make_in_maps(x, W_q, W_k, W_v, W_o)
    res = run_bass_kernel_spmd(nc, in_maps, core_ids=list(range(8)))

    out = np.empty((4, N_SEQ, D_EMB), np.float32)
    for b in range(4):
        out[b] = (
            res.results[2 * b]["out"]
            + res.results[2 * b + 1]["out"]
            + b_o[None, :].astype(np.float32)
        )
    return out



# revision 3
# speedup vs baseline: 1.4740x; 1.4740x over previous
"""Causal self-attention Trainium2 kernel (8 NeuronCores).

Sharding (Megatron-style, per sharding_hint):
  core c -> batch b = c//2, head-group g = c%2 (8 of 16 heads).
  W_q/W_k/W_v column-sliced per head group; W_o row-sliced; host sums the
  two partial outputs per batch (tensor-parallel reduce) and adds b_o.

Per-core kernel (all matmuls bf16 with fp32 PSUM accumulation):
  xT    [1024, 2048]  x[b] transposed (d_emb on partitions)
  wqkv  [1024, 1536]  [Wq_g | Wk_g | Wv_g]
  wo    [512, 1024]   W_o rows for this head group
  out   [2048, 1024]  fp32 partial (no bias)

Layouts: qT/kT stored [head_dim, n] so score matmuls contract over the
64-dim head axis; two heads per 128-partition tile (partitions 0:64 /
64:128). Scores are computed TRANSPOSED ([k, q]) so the exp'd weights
feed the ctx matmul directly as the moving operand; v is kept
[n, head_dim] with a ones-block per head so a single matmul yields both
ctx^T and the softmax denominators.

Pipeline structure (the point of this version): ALL pools live for the
whole body — no scoped pool reuse, so there are no WAR deps forcing the
projection phase to drain before attention starts. Emission is
chunk-pipelined: attention on query-chunk qc is interleaved with the
q/k/v projections for chunk qc+1 and (in the last chunk) the deferred
out-projections, so the PE always has independent work while the scalar
engine chews through the softmax exps (the second-busiest engine).
The exp is trimmed to the causal region where the trim pays for the
extra instruction.

PSUM budget (8 banks): pj 2x[128,512] (proj + out-proj) = 2, sc
2x[128,1024] (scores) = 4, cx 2x[128,512] (ctx accum) = 2.

`reps` repeats the whole body inside one NEFF — used only for timing
((T(n)-T(1))/(n-1) cancels dispatch overhead); the graded path is reps=1.
"""

import sys

import numpy as np

sys.path.insert(0, "/opt/trn_rl_repo")

import ml_dtypes

BF16 = ml_dtypes.bfloat16

D_EMB = 1024
N_SEQ = 2048
N_HEADS_CORE = 8  # heads per core
HD = 64  # head dim
KT = D_EMB // 128  # 8 k-tiles over d_emb
PT = 4  # partition tiles over the 512 per-core head dims
NT = N_SEQ // 128  # 16 n-tiles
QC = N_SEQ // 512  # 4 query chunks of 512
SCALE = 1.0 / np.sqrt(HD)

_CACHE = {}


def _emit_body(nc, tc, mybir, sfx, xT_d, wqkv_d, wo_d, out_d):
    f32 = mybir.dt.float32
    bf16 = mybir.dt.bfloat16

    with (
        tc.tile_pool(name=f"persist{sfx}", bufs=1) as persist,
        tc.tile_pool(name=f"expp{sfx}", bufs=6) as expp,
        tc.tile_pool(name=f"rpool{sfx}", bufs=4) as rpool,
        tc.tile_pool(name=f"outp{sfx}", bufs=3) as outp,
        tc.tile_pool(name=f"pj{sfx}", bufs=2, space="PSUM") as pj_pool,
        tc.tile_pool(name=f"sc{sfx}", bufs=2, space="PSUM") as sc_pool,
        tc.tile_pool(name=f"cx{sfx}", bufs=2, space="PSUM") as cx_pool,
    ):
        xt_sb = [
            persist.tile([128, N_SEQ], bf16, name=f"xt{k}{sfx}", tag=f"xt{k}")
            for k in range(KT)
        ]
        wqkv_sb = [
            persist.tile([128, 1536], bf16, name=f"wqkv{k}{sfx}", tag=f"wqkv{k}")
            for k in range(KT)
        ]
        wo_sb = [
            persist.tile([128, D_EMB], bf16, name=f"wo{p}{sfx}", tag=f"wo{p}")
            for p in range(PT)
        ]
        qt_sb = [
            persist.tile([128, N_SEQ], bf16, name=f"qt{p}{sfx}", tag=f"qt{p}")
            for p in range(PT)
        ]
        kt_sb = [
            persist.tile([128, N_SEQ], bf16, name=f"kt{p}{sfx}", tag=f"kt{p}")
            for p in range(PT)
        ]
        ctxt_sb = [
            persist.tile([128, N_SEQ], bf16, name=f"ctxt{p}{sfx}", tag=f"ctxt{p}")
            for p in range(PT)
        ]
        # v per n-tile [128, 1024]: head h -> cols h*128:h*128+64 = v_h,
        # cols h*128+64:h*128+128 = 1.0 (softmax denominator ones-trick)
        v_sb = [
            persist.tile([128, 1024], bf16, name=f"v{nt}{sfx}", tag=f"v{nt}")
            for nt in range(NT)
        ]
        tri_sb = persist.tile([128, 128], bf16, name=f"tri{sfx}", tag="tri")

        def vaug_ap(nt, h):
            return v_sb[nt][:, h * 128 : (h + 1) * 128]

        # ---- constants (gpsimd, no deps) ----
        for nt in range(NT):
            ones_view = v_sb[nt].rearrange("p (h c) -> p h c", h=N_HEADS_CORE)
            nc.gpsimd.memset(ones_view[:, :, 64:128], 1.0)
        # tri[k_local, q_local] = 1.0 if q_local >= k_local else 0
        nc.gpsimd.memset(tri_sb[:], 1.0)
        nc.gpsimd.affine_select(
            out=tri_sb[:],
            in_=tri_sb[:],
            compare_op=mybir.AluOpType.is_ge,
            fill=0.0,
            base=0,
            pattern=[[1, 128]],
            channel_multiplier=-1,
        )

        # ---- input DMAs, split so early consumers unblock early ----
        # wqkv per (k, group): v group first (first proj units), then k, q.
        for grp in (2, 1, 0):
            for k in range(KT):
                nc.sync.dma_start(
                    out=wqkv_sb[k][:, grp * 512 : (grp + 1) * 512],
                    in_=wqkv_d[k * 128 : (k + 1) * 128, grp * 512 : (grp + 1) * 512],
                )
        # xT per (k, half): first halves first.
        for h in range(2):
            for k in range(KT):
                nc.gpsimd.dma_start(
                    out=xt_sb[k][:, h * 1024 : (h + 1) * 1024],
                    in_=xT_d[k * 128 : (k + 1) * 128, h * 1024 : (h + 1) * 1024],
                )
        for p in range(PT):
            nc.sync.dma_start(out=wo_sb[p][:], in_=wo_d[p * 128 : (p + 1) * 128, :])

        # ---- projection / out-projection emitters (interleavable units) --
        def emit_v(nt):
            psv = pj_pool.tile([128, 512], f32, name=f"psv{nt}{sfx}", tag="pj")
            for k in range(KT):
                nc.tensor.matmul(
                    psv[:],
                    lhsT=xt_sb[k][:, nt * 128 : (nt + 1) * 128],
                    rhs=wqkv_sb[k][:, 1024:1536],
                    start=(k == 0),
                    stop=(k == KT - 1),
                )
            v_view = v_sb[nt].rearrange("p (h c) -> p h c", h=N_HEADS_CORE)
            nc.vector.tensor_copy(
                v_view[:, :, 0:64],
                psv.rearrange("p (h c) -> p h c", h=N_HEADS_CORE),
            )

        def emit_qk(qn, p, which):
            # which: 0 = q, 1 = k
            nsl = slice(qn * 512, (qn + 1) * 512)
            ps = pj_pool.tile(
                [128, 512], f32, name=f"psqk{p}_{qn}_{which}{sfx}", tag="pj"
            )
            col0 = which * 512 + p * 128
            for k in range(KT):
                nc.tensor.matmul(
                    ps[:],
                    lhsT=wqkv_sb[k][:, col0 : col0 + 128],
                    rhs=xt_sb[k][:, nsl],
                    start=(k == 0),
                    stop=(k == KT - 1),
                )
            dst = kt_sb if which else qt_sb
            nc.vector.tensor_copy(dst[p][:, nsl], ps[:])

        def emit_outproj(nt, dh):
            pso = pj_pool.tile([128, 512], f32, name=f"pso{nt}_{dh}{sfx}", tag="pj")
            csl = slice(dh * 512, (dh + 1) * 512)
            for p in range(PT):
                nc.tensor.matmul(
                    pso[:],
                    lhsT=ctxt_sb[p][:, nt * 128 : (nt + 1) * 128],
                    rhs=wo_sb[p][:, csl],
                    start=(p == 0),
                    stop=(p == PT - 1),
                )
            osb = outp.tile([128, 512], f32, name=f"osb{nt}_{dh}{sfx}", tag="osb")
            nc.vector.tensor_copy(osb[:], pso[:])
            nc.sync.dma_start(
                out=out_d[nt * 128 : (nt + 1) * 128, csl], in_=osb[:]
            )

        # ---- attention for one (qc, p) pair ----
        def emit_attn_p(qc, p):
            q0 = qc * 512
            nk = 4 * qc + 4  # causal: k-tiles 0..nk-1
            ngroups = nk // 2
            cx = [
                cx_pool.tile([128, 512], f32, name=f"cx{p}_{qc}_{h2}{sfx}", tag="cx")
                for h2 in range(2)
            ]
            for gi in range(ngroups):
                ps = [
                    sc_pool.tile(
                        [128, 1024], f32, name=f"sc{p}_{qc}_{gi}_{h2}{sfx}", tag="sc"
                    )
                    for h2 in range(2)
                ]
                ex = [
                    expp.tile(
                        [128, 1024], bf16, name=f"ex{p}_{qc}_{gi}_{h2}{sfx}", tag="ex"
                    )
                    for h2 in range(2)
                ]
                t0s = []
                for j in range(2):
                    ki = 2 * gi + j
                    jj = ki - 4 * qc  # >=0 on diagonal sub-tiles
                    t0s.append(max(0, 128 * jj))
                # interleave heads' score matmuls
                for j in range(2):
                    ki, t0 = 2 * gi + j, t0s[j]
                    for h2 in range(2):
                        hb = h2 * 64
                        nc.tensor.matmul(
                            ps[h2][:, j * 512 + t0 : (j + 1) * 512],
                            lhsT=kt_sb[p][hb : hb + 64, ki * 128 : (ki + 1) * 128],
                            rhs=qt_sb[p][hb : hb + 64, q0 + t0 : q0 + 512],
                            start=True,
                            stop=True,
                        )
                # exp (split per j on diagonal groups so only written PSUM
                # columns are ever read)
                split = (t0s[0] + t0s[1]) > 0
                for h2 in range(2):
                    if split:
                        for j in range(2):
                            t0 = t0s[j]
                            blk = slice(j * 512 + t0, (j + 1) * 512)
                            nc.scalar.activation(
                                ex[h2][:, blk],
                                ps[h2][:, blk],
                                mybir.ActivationFunctionType.Exp,
                                scale=float(SCALE),
                            )
                    else:
                        nc.scalar.activation(
                            ex[h2][:],
                            ps[h2][:],
                            mybir.ActivationFunctionType.Exp,
                            scale=float(SCALE),
                        )
                # triangular mask on diagonal blocks
                for j in range(2):
                    ki = 2 * gi + j
                    jj = ki - 4 * qc
                    if jj >= 0:
                        blk = slice(j * 512 + 128 * jj, j * 512 + 128 * jj + 128)
                        for h2 in range(2):
                            nc.vector.tensor_mul(
                                ex[h2][:, blk], ex[h2][:, blk], tri_sb[:]
                            )
                # ctx accumulation
                for j in range(2):
                    ki, t0 = 2 * gi + j, t0s[j]
                    for h2 in range(2):
                        h = 2 * p + h2
                        nc.tensor.matmul(
                            cx[h2][:, t0:512],
                            lhsT=vaug_ap(ki, h),
                            rhs=ex[h2][:, j * 512 + t0 : (j + 1) * 512],
                            start=(ki == 0),
                            stop=(ki == nk - 1),
                        )
            # drain: normalize ctx^T by the softmax denominators
            for h2 in range(2):
                rec = rpool.tile(
                    [64, 512], f32, name=f"rec{p}_{qc}_{h2}{sfx}", tag="rec"
                )
                nc.vector.reciprocal(rec[:], cx[h2][64:128, :])
                nc.vector.tensor_mul(
                    ctxt_sb[p][h2 * 64 : h2 * 64 + 64, q0 : q0 + 512],
                    cx[h2][0:64, :],
                    rec[:],
                )

        # ---- chunk-pipelined emission ----
        # prologue: projections for chunk 0
        for nt in range(4):
            emit_v(nt)
        for p in range(PT):
            emit_qk(0, p, 1)
        for p in range(PT):
            emit_qk(0, p, 0)

        for qc in range(QC):
            # filler units PE can chew while ACT runs this chunk's exps
            filler = []
            if qc < QC - 1:
                cn = qc + 1
                filler += [(emit_v, (nt,)) for nt in range(4 * cn, 4 * cn + 4)]
                filler += [(emit_qk, (cn, p, 1)) for p in range(PT)]
                filler += [(emit_qk, (cn, p, 0)) for p in range(PT)]
            else:
                # deferred out-projections of chunks 0..2
                filler += [
                    (emit_outproj, (nt, dh)) for nt in range(12) for dh in range(2)
                ]
            nf = len(filler)
            fi = 0
            for p in range(PT):
                emit_attn_p(qc, p)
                # spread filler roughly evenly across the 4 p-blocks
                target = (p + 1) * nf // PT
                while fi < target:
                    fn, args = filler[fi]
                    fn(*args)
                    fi += 1
        # tail: out-projection of the last chunk
        for nt in range(12, 16):
            for dh in range(2):
                emit_outproj(nt, dh)


def _build_module(reps=1):
    import concourse.bacc as bacc
    import concourse.mybir as mybir
    import concourse.tile as tile

    f32 = mybir.dt.float32
    bf16 = mybir.dt.bfloat16

    nc = bacc.Bacc()
    xT_d = nc.dram_tensor("xT", [D_EMB, N_SEQ], bf16, kind="ExternalInput")
    wqkv_d = nc.dram_tensor("wqkv", [D_EMB, 1536], bf16, kind="ExternalInput")
    wo_d = nc.dram_tensor("wo", [512, D_EMB], bf16, kind="ExternalInput")
    out_d = nc.dram_tensor("out", [N_SEQ, D_EMB], f32, kind="ExternalOutput")

    with tile.TileContext(nc) as tc:
        for rep in range(reps):
            _emit_body(
                nc, tc, mybir, f"_r{rep}" if reps > 1 else "",
                xT_d, wqkv_d, wo_d, out_d,
            )

    if not nc.is_finalized():
        nc.finalize()
    return nc


def _get_module(reps=1):
    key = f"nc{reps}"
    if key not in _CACHE:
        _CACHE[key] = _build_module(reps)
    return _CACHE[key]


def make_in_maps(x, W_q, W_k, W_v, W_o):
    in_maps = []
    for c in range(8):
        b, g = c // 2, c % 2
        gs = slice(g * 512, (g + 1) * 512)
        xT = np.ascontiguousarray(x[b].T).astype(BF16)
        wqkv = np.concatenate(
            [W_q[:, gs], W_k[:, gs], W_v[:, gs]], axis=1
        ).astype(BF16)
        wo = np.ascontiguousarray(W_o[gs, :]).astype(BF16)
        in_maps.append({"xT": xT, "wqkv": wqkv, "wo": wo})
    return in_maps


def kernel(x, W_q, W_k, W_v, W_o, b_o):
    from concourse.bass_utils import run_bass_kernel_spmd

    nc = _get_module()
    in_maps = make_in_maps(x, W_q, W_k, W_v, W_o)
    res = run_bass_kernel_spmd(nc, in_maps, core_ids=list(range(8)))

    out = np.empty((4, N_SEQ, D_EMB), np.float32)
    for b in range(4):
        out[b] = (
            res.results[2 * b]["out"]
            + res.results[2 * b + 1]["out"]
            + b_o[None, :].astype(np.float32)
        )
    return out


# revision 5
# speedup vs baseline: 1.6554x; 1.1231x over previous
"""Causal self-attention Trainium2 kernel (8 NeuronCores).

Sharding (Megatron-style, per sharding_hint):
  core c -> batch b = c//2, head-group g = c%2 (8 of 16 heads).
  W_q/W_k/W_v column-sliced per head group; W_o row-sliced; host sums the
  two partial outputs per batch (tensor-parallel reduce) and adds b_o.

Per-core kernel (all matmuls bf16 with fp32 PSUM accumulation):
  xT    [1024, 2048]  x[b] transposed (d_emb on partitions)
  wqkv  [1024, 1536]  [Wq_g | Wk_g | Wv_g]
  wo    [512, 1024]   W_o rows for this head group
  out   [2048, 1024]  fp32 partial (no bias)

Layouts: qT/kT stored [head_dim, n] so score matmuls contract over the
64-dim head axis; two heads per 128-partition tile (partitions 0:64 /
64:128). Scores are computed TRANSPOSED ([k, q]) so the exp'd weights
feed the ctx matmul directly as the moving operand; v is kept
[n, head_dim] with a ones-block per head so a single matmul yields both
ctx^T and the softmax denominators.

Pipeline structure (the point of this version): ALL pools live for the
whole body — no scoped pool reuse, so there are no WAR deps forcing the
projection phase to drain before attention starts. Emission is
chunk-pipelined: attention on query-chunk qc is interleaved with the
q/k/v projections for chunk qc+1 and (in the last chunk) the deferred
out-projections, so the PE always has independent work while the scalar
engine chews through the softmax exps (the second-busiest engine).
The exp is trimmed to the causal region where the trim pays for the
extra instruction.

PSUM budget (8 banks): pj 2x[128,512] (proj + out-proj) = 2, sc
2x[128,1024] (scores) = 4, cx 2x[128,512] (ctx accum) = 2.

`reps` repeats the whole body inside one NEFF — used only for timing
((T(n)-T(1))/(n-1) cancels dispatch overhead); the graded path is reps=1.
"""

import sys

import numpy as np

sys.path.insert(0, "/opt/trn_rl_repo")

import ml_dtypes

BF16 = ml_dtypes.bfloat16

D_EMB = 1024
N_SEQ = 2048
N_HEADS_CORE = 8  # heads per core
HD = 64  # head dim
KT = D_EMB // 128  # 8 k-tiles over d_emb
PT = 4  # partition tiles over the 512 per-core head dims
NT = N_SEQ // 128  # 16 n-tiles
QC = N_SEQ // 512  # 4 query chunks of 512
SCALE = 1.0 / np.sqrt(HD)

_CACHE = {}


def _emit_body(nc, tc, mybir, sfx, xT_d, wqkv_d, wo_d, out_d):
    f32 = mybir.dt.float32
    bf16 = mybir.dt.bfloat16

    with (
        tc.tile_pool(name=f"persist{sfx}", bufs=1) as persist,
        tc.tile_pool(name=f"expp{sfx}", bufs=6) as expp,
        tc.tile_pool(name=f"rpool{sfx}", bufs=4) as rpool,
        tc.tile_pool(name=f"outp{sfx}", bufs=3) as outp,
        tc.tile_pool(name=f"pj{sfx}", bufs=2, space="PSUM") as pj_pool,
        tc.tile_pool(name=f"sc{sfx}", bufs=2, space="PSUM") as sc_pool,
        tc.tile_pool(name=f"cx{sfx}", bufs=2, space="PSUM") as cx_pool,
    ):
        xt_sb = [
            persist.tile([128, N_SEQ], bf16, name=f"xt{k}{sfx}", tag=f"xt{k}")
            for k in range(KT)
        ]
        wqkv_sb = [
            persist.tile([128, 1536], bf16, name=f"wqkv{k}{sfx}", tag=f"wqkv{k}")
            for k in range(KT)
        ]
        wo_sb = [
            persist.tile([128, D_EMB], bf16, name=f"wo{p}{sfx}", tag=f"wo{p}")
            for p in range(PT)
        ]
        qt_sb = [
            persist.tile([128, N_SEQ], bf16, name=f"qt{p}{sfx}", tag=f"qt{p}")
            for p in range(PT)
        ]
        kt_sb = [
            persist.tile([128, N_SEQ], bf16, name=f"kt{p}{sfx}", tag=f"kt{p}")
            for p in range(PT)
        ]
        ctxt_sb = [
            persist.tile([128, N_SEQ], bf16, name=f"ctxt{p}{sfx}", tag=f"ctxt{p}")
            for p in range(PT)
        ]
        # v per n-tile [128, 1024]: head h -> cols h*128:h*128+64 = v_h,
        # cols h*128+64:h*128+128 = 1.0 (softmax denominator ones-trick)
        v_sb = [
            persist.tile([128, 1024], bf16, name=f"v{nt}{sfx}", tag=f"v{nt}")
            for nt in range(NT)
        ]
        tri_sb = persist.tile([128, 128], bf16, name=f"tri{sfx}", tag="tri")

        def vaug_ap(nt, h):
            return v_sb[nt][:, h * 128 : (h + 1) * 128]

        # ---- input DMAs, split so early consumers unblock early ----
        # xT per (k, half) on the SP HWDGE queue: first halves first.
        for h in range(2):
            for k in range(KT):
                nc.sync.dma_start(
                    out=xt_sb[k][:, h * 1024 : (h + 1) * 1024],
                    in_=xT_d[k * 128 : (k + 1) * 128, h * 1024 : (h + 1) * 1024],
                )
        # wqkv per (k, group) on the ACT HWDGE queue (ACT idle early):
        # v group first (first proj units), then k, q.
        for grp in (2, 1, 0):
            for k in range(KT):
                nc.scalar.dma_start(
                    out=wqkv_sb[k][:, grp * 512 : (grp + 1) * 512],
                    in_=wqkv_d[k * 128 : (k + 1) * 128, grp * 512 : (grp + 1) * 512],
                )
        for p in range(PT):
            nc.scalar.dma_start(out=wo_sb[p][:], in_=wo_d[p * 128 : (p + 1) * 128, :])

        # ---- constants ----
        # ones blocks needed by chunk-0 ctx go on DVE (idle at start);
        # the rest on gpsimd, which has nothing else to do.
        for nt in range(NT):
            ones_view = v_sb[nt].rearrange("p (h c) -> p h c", h=N_HEADS_CORE)
            eng = nc.vector if nt < 4 else nc.gpsimd
            eng.memset(ones_view[:, :, 64:128], 1.0)
        # tri[k_local, q_local] = 1.0 if q_local >= k_local else 0
        nc.gpsimd.memset(tri_sb[:], 1.0)
        nc.gpsimd.affine_select(
            out=tri_sb[:],
            in_=tri_sb[:],
            compare_op=mybir.AluOpType.is_ge,
            fill=0.0,
            base=0,
            pattern=[[1, 128]],
            channel_multiplier=-1,
        )

        # ---- projection / out-projection emitters (interleavable units) --
        def emit_v(nt):
            psv = pj_pool.tile([128, 512], f32, name=f"psv{nt}{sfx}", tag="pj")
            for k in range(KT):
                nc.tensor.matmul(
                    psv[:],
                    lhsT=xt_sb[k][:, nt * 128 : (nt + 1) * 128],
                    rhs=wqkv_sb[k][:, 1024:1536],
                    start=(k == 0),
                    stop=(k == KT - 1),
                )
            v_view = v_sb[nt].rearrange("p (h c) -> p h c", h=N_HEADS_CORE)
            nc.vector.tensor_copy(
                v_view[:, :, 0:64],
                psv.rearrange("p (h c) -> p h c", h=N_HEADS_CORE),
            )

        def emit_qk(qn, p, which):
            # which: 0 = q, 1 = k
            nsl = slice(qn * 512, (qn + 1) * 512)
            ps = pj_pool.tile(
                [128, 512], f32, name=f"psqk{p}_{qn}_{which}{sfx}", tag="pj"
            )
            col0 = which * 512 + p * 128
            for k in range(KT):
                nc.tensor.matmul(
                    ps[:],
                    lhsT=wqkv_sb[k][:, col0 : col0 + 128],
                    rhs=xt_sb[k][:, nsl],
                    start=(k == 0),
                    stop=(k == KT - 1),
                )
            dst = kt_sb if which else qt_sb
            nc.vector.tensor_copy(dst[p][:, nsl], ps[:])

        def emit_outproj(nt, dh):
            pso = pj_pool.tile([128, 512], f32, name=f"pso{nt}_{dh}{sfx}", tag="pj")
            csl = slice(dh * 512, (dh + 1) * 512)
            for p in range(PT):
                nc.tensor.matmul(
                    pso[:],
                    lhsT=ctxt_sb[p][:, nt * 128 : (nt + 1) * 128],
                    rhs=wo_sb[p][:, csl],
                    start=(p == 0),
                    stop=(p == PT - 1),
                )
            osb = outp.tile([128, 512], f32, name=f"osb{nt}_{dh}{sfx}", tag="osb")
            nc.vector.tensor_copy(osb[:], pso[:])
            nc.sync.dma_start(
                out=out_d[nt * 128 : (nt + 1) * 128, csl], in_=osb[:]
            )

        # ---- attention for one (qc, p) pair ----
        def emit_attn_p(qc, p):
            q0 = qc * 512
            nk = 4 * qc + 4  # causal: k-tiles 0..nk-1
            ngroups = nk // 2
            cx = [
                cx_pool.tile([128, 512], f32, name=f"cx{p}_{qc}_{h2}{sfx}", tag="cx")
                for h2 in range(2)
            ]
            for gi in range(ngroups):
                ps = [
                    sc_pool.tile(
                        [128, 1024], f32, name=f"sc{p}_{qc}_{gi}_{h2}{sfx}", tag="sc"
                    )
                    for h2 in range(2)
                ]
                ex = [
                    expp.tile(
                        [128, 1024], bf16, name=f"ex{p}_{qc}_{gi}_{h2}{sfx}", tag="ex"
                    )
                    for h2 in range(2)
                ]
                t0s = []
                for j in range(2):
                    ki = 2 * gi + j
                    jj = ki - 4 * qc  # >=0 on diagonal sub-tiles
                    t0s.append(max(0, 128 * jj))
                # interleave heads' score matmuls
                for j in range(2):
                    ki, t0 = 2 * gi + j, t0s[j]
                    for h2 in range(2):
                        hb = h2 * 64
                        nc.tensor.matmul(
                            ps[h2][:, j * 512 + t0 : (j + 1) * 512],
                            lhsT=kt_sb[p][hb : hb + 64, ki * 128 : (ki + 1) * 128],
                            rhs=qt_sb[p][hb : hb + 64, q0 + t0 : q0 + 512],
                            start=True,
                            stop=True,
                        )
                # exp (split per j on diagonal groups so only written PSUM
                # columns are ever read)
                split = (t0s[0] + t0s[1]) > 0
                for h2 in range(2):
                    if split:
                        for j in range(2):
                            t0 = t0s[j]
                            blk = slice(j * 512 + t0, (j + 1) * 512)
                            nc.scalar.activation(
                                ex[h2][:, blk],
                                ps[h2][:, blk],
                                mybir.ActivationFunctionType.Exp,
                                scale=float(SCALE),
                            )
                    else:
                        nc.scalar.activation(
                            ex[h2][:],
                            ps[h2][:],
                            mybir.ActivationFunctionType.Exp,
                            scale=float(SCALE),
                        )
                # triangular mask on diagonal blocks
                for j in range(2):
                    ki = 2 * gi + j
                    jj = ki - 4 * qc
                    if jj >= 0:
                        blk = slice(j * 512 + 128 * jj, j * 512 + 128 * jj + 128)
                        for h2 in range(2):
                            nc.vector.tensor_mul(
                                ex[h2][:, blk], ex[h2][:, blk], tri_sb[:]
                            )
                # ctx accumulation
                for j in range(2):
                    ki, t0 = 2 * gi + j, t0s[j]
                    for h2 in range(2):
                        h = 2 * p + h2
                        nc.tensor.matmul(
                            cx[h2][:, t0:512],
                            lhsT=vaug_ap(ki, h),
                            rhs=ex[h2][:, j * 512 + t0 : (j + 1) * 512],
                            start=(ki == 0),
                            stop=(ki == nk - 1),
                        )
            # drain: normalize ctx^T by the softmax denominators
            for h2 in range(2):
                rec = rpool.tile(
                    [64, 512], f32, name=f"rec{p}_{qc}_{h2}{sfx}", tag="rec"
                )
                nc.vector.reciprocal(rec[:], cx[h2][64:128, :])
                nc.vector.tensor_mul(
                    ctxt_sb[p][h2 * 64 : h2 * 64 + 64, q0 : q0 + 512],
                    cx[h2][0:64, :],
                    rec[:],
                )

        # ---- chunk-pipelined emission ----
        # prologue: projections for chunk 0
        for nt in range(4):
            emit_v(nt)
        for p in range(PT):
            emit_qk(0, p, 1)
        for p in range(PT):
            emit_qk(0, p, 0)

        for qc in range(QC):
            # filler units PE can chew while ACT runs this chunk's exps
            filler = []
            if qc < QC - 1:
                cn = qc + 1
                filler += [(emit_v, (nt,)) for nt in range(4 * cn, 4 * cn + 4)]
                filler += [(emit_qk, (cn, p, 1)) for p in range(PT)]
                filler += [(emit_qk, (cn, p, 0)) for p in range(PT)]
            else:
                # deferred out-projections of chunks 0..2
                filler += [
                    (emit_outproj, (nt, dh)) for nt in range(12) for dh in range(2)
                ]
            nf = len(filler)
            fi = 0
            for p in range(PT):
                emit_attn_p(qc, p)
                if qc < QC - 1:
                    # spread filler roughly evenly across the 4 p-blocks
                    target = (p + 1) * nf // PT
                else:
                    # back-load the last chunk: keep half the out-proj units
                    # after the final p-block so the PE has work while ACT
                    # finishes the last exps
                    target = (p + 1) * nf // (2 * PT)
                while fi < target:
                    fn, args = filler[fi]
                    fn(*args)
                    fi += 1
            while fi < nf:
                fn, args = filler[fi]
                fn(*args)
                fi += 1
        # tail: out-projection of the last chunk
        for nt in range(12, 16):
            for dh in range(2):
                emit_outproj(nt, dh)


def _build_module(reps=1):
    import concourse.bacc as bacc
    import concourse.mybir as mybir
    import concourse.tile as tile

    f32 = mybir.dt.float32
    bf16 = mybir.dt.bfloat16

    nc = bacc.Bacc()
    xT_d = nc.dram_tensor("xT", [D_EMB, N_SEQ], bf16, kind="ExternalInput")
    wqkv_d = nc.dram_tensor("wqkv", [D_EMB, 1536], bf16, kind="ExternalInput")
    wo_d = nc.dram_tensor("wo", [512, D_EMB], bf16, kind="ExternalInput")
    out_d = nc.dram_tensor("out", [N_SEQ, D_EMB], f32, kind="ExternalOutput")

    with tile.TileContext(nc) as tc:
        for rep in range(reps):
            _emit_body(
                nc, tc, mybir, f"_r{rep}" if reps > 1 else "",
                xT_d, wqkv_d, wo_d, out_d,
            )

    if not nc.is_finalized():
        nc.finalize()
    return nc


def _get_module(reps=1):
    key = f"nc{reps}"
    if key not in _CACHE:
        _CACHE[key] = _build_module(reps)
    return _CACHE[key]


def make_in_maps(x, W_q, W_k, W_v, W_o):
    in_maps = []
    for c in range(8):
        b, g = c // 2, c % 2
        gs = slice(g * 512, (g + 1) * 512)
        xT = np.ascontiguousarray(x[b].T).astype(BF16)
        wqkv = np.concatenate(
            [W_q[:, gs], W_k[:, gs], W_v[:, gs]], axis=1
        ).astype(BF16)
        wo = np.ascontiguousarray(W_o[gs, :]).astype(BF16)
        in_maps.append({"xT": xT, "wqkv": wqkv, "wo": wo})
    return in_maps


def kernel(x, W_q, W_k, W_v, W_o, b_o):
    from concourse.bass_utils import run_bass_kernel_spmd

    nc = _get_module()
    in_maps = make_in_maps(x, W_q, W_k, W_v, W_o)
    res = run_bass_kernel_spmd(nc, in_maps, core_ids=list(range(8)))

    out = np.empty((4, N_SEQ, D_EMB), np.float32)
    for b in range(4):
        out[b] = (
            res.results[2 * b]["out"]
            + res.results[2 * b + 1]["out"]
            + b_o[None, :].astype(np.float32)
        )
    return out
